# revision 57
# baseline (speedup 1.0000x reference)
"""Trainium2 Bass kernel for the FNO-style spectral layer.

Math: reference computes y = irfft(rfft(x) + delta) along L where delta
only touches output bins 0..63:
    delta[k] = fre[index[k]] * wr[k] + i * fim[index[k]] * wi[k]
By linearity of rfft/irfft, y = x + x @ P @ Q where
    P[n, k]      =  wr[k] * cos(2*pi*index[k]*n/L) / sqrt(L)
    P[n, 64+k]   = -wi[k] * sin(2*pi*index[k]*n/L) / sqrt(L)
    Q[k, n]      =  c_k * cos(2*pi*k*n/L)          (c_0 = 1/sqrt(L), else 2/sqrt(L))
    Q[64+k, n]   = -c_k * sin(2*pi*k*n/L)
(the jax irfft ignores the imaginary part of bin 0; row 64 of Q is zero
anyway since sin(0) == 0).

The norm rel-err budget (2e-2) is ~10x above bf16 I/O noise (~3e-3), so
everything runs in bf16: HBM traffic halves versus f32 (the kernel is
memory-bound — 45 MB/core, 90 MB per core-pair through one 716 GB/s
stack -> ~126 us floor vs ~255 us for f32).

The host uploads x PRE-TRANSPOSED per core as [NT, 4, 125, 2, RB] where
element (t, c4, p, c2, r) = x_rows[t*RB + r, (2*c4 + c2)*125 + p]. The
device then needs NO PE transposes at all:
    MM1: A^T[2m, RB] = sum_c P_c^T @ xt_c      (contract L in 8 chunks)
    MM2: corr^T chunk c = Q_c^T @ A^T          (per 125-row L chunk)
    y^T = x^T + corr^T   (5 chunks: DVE adds from PSUM; 3 chunks:
                          ACT copy PSUM->SBUF + GpSimd SBUF add)
y is stored in the same transposed tiled layout and un-permuted on the
host. The partition dim is PADDED 125 -> 128 (zero rows): the DGE
splits a DMA instruction's partition dim EVENLY across DMA engines, so
the engine count is the largest divisor of the partition count <= 16.
125 partitions -> 5 engines -> 1/3 bandwidth (measured, twice); 128
partitions -> all 16. The (4, 2) chunk split keeps descriptors at 2 KB,
512 per instruction.

Measured: ~140-146 us HW exec per 8-core run (vs 274 us for the f32
version of the same pipeline), rel err 2.3e-03 against the f32 jax
reference. DMA engines sit at ~83-85% of the 46 MB / 358 GB/s per-core
floor (~129 us); the rest is NEFF startup (~5.5 us), the end-of-kernel
semaphore drain (~4 us), and drain-phase gaps where the last tiles'
stores trail their adds. Run-to-run spread is a few us.
"""

import sys

if "/opt/trn_rl_repo" not in sys.path:
    sys.path.insert(0, "/opt/trn_rl_repo")

import ml_dtypes
import numpy as np

import concourse.bass as bass  # noqa: F401  (kept for AP helpers)
import concourse.mybir as mybir
from concourse import bacc
from concourse.bass_utils import run_bass_kernel_spmd
from concourse.masks import make_identity
from concourse.tile import TileContext

B, E, L = 4096, 22, 1000
MODES = 64
M2 = 2 * MODES                # 128
NCORES = 8
ROWS = B * E                  # 90112
R_CORE = ROWS // NCORES       # 11264
RB = 512                      # batch-rows per tile
NT = R_CORE // RB             # 22
KC = 125                      # L-chunk (partition dim), 8 * 125 = 1000
NCH = L // KC                 # 8

KP = 128                      # padded partition dim (KC zero-padded)

F32 = mybir.dt.float32
BF16 = mybir.dt.bfloat16
NP_BF16 = ml_dtypes.bfloat16

# knobs (module-level so test.py can flip them before first kernel() call)
TRACE = False
LAST_RESULT = None


def _build_pq(fweights, fweights_im, index):
    """Host-side: analysis P [L, 2m] and synthesis Q [2m, L] in float64."""
    fw = np.asarray(fweights, dtype=np.float64)
    fwi = np.asarray(fweights_im, dtype=np.float64)
    idx = np.asarray(index, dtype=np.int64)
    m = idx.shape[0]
    widx = np.concatenate([[0], np.arange(1, m) + 1])
    wr = fw[widx, 0]
    wi = fwi[widx, 0]
    n = np.arange(L, dtype=np.float64)
    ang_in = 2.0 * np.pi * np.outer(n, idx.astype(np.float64)) / L
    P = np.zeros((L, 2 * m), dtype=np.float64)
    P[:, :m] = np.cos(ang_in) * wr / np.sqrt(L)
    P[:, m:] = -np.sin(ang_in) * wi / np.sqrt(L)
    k_out = np.arange(m, dtype=np.float64)
    ang_out = 2.0 * np.pi * np.outer(k_out, n) / L
    c = np.full(m, 2.0 / np.sqrt(L))
    c[0] = 1.0 / np.sqrt(L)
    Q = np.zeros((2 * m, L), dtype=np.float64)
    Q[:m, :] = np.cos(ang_out) * c[:, None]
    Q[m:, :] = -np.sin(ang_out) * c[:, None]
    return P, Q


_nc_cache = None


def _build_bass():
    nc = bacc.Bacc(None, target_bir_lowering=False)
    x_d = nc.dram_tensor(
        "x", [NT, NCH // 2, KP, 2, RB], BF16, kind="ExternalInput"
    )
    p_d = nc.dram_tensor("p", [KC, NCH, M2], BF16, kind="ExternalInput")
    q_d = nc.dram_tensor("q", [M2, NCH, KP], BF16, kind="ExternalInput")
    y_d = nc.dram_tensor(
        "y", [NT, NCH // 2, KP, 2, RB], BF16, kind="ExternalOutput"
    )

    with TileContext(nc) as tc:
        with (
            tc.tile_pool(name="consts", bufs=1) as consts,
            tc.tile_pool(name="xin", bufs=8) as xin,
            tc.tile_pool(name="apool", bufs=3) as apool,
            tc.tile_pool(name="yout", bufs=4) as yout,
            tc.tile_pool(name="ps_a", bufs=2, space="PSUM") as ps_a,
            tc.tile_pool(name="ps_c", bufs=6, space="PSUM") as ps_c,
        ):
            # params staged on the SWDGE (gpsimd) ring so the SP ring is
            # free for the first x loads
            pP = consts.tile([KC, NCH, M2], BF16)
            nc.gpsimd.dma_start(out=pP, in_=p_d[:, :, :])
            qQ = consts.tile([M2, NCH, KP], BF16)
            nc.gpsimd.dma_start(out=qQ, in_=q_d[:, :, :])
            ident = consts.tile([KP, KP], BF16)
            make_identity(nc, ident)

            for t in range(NT):
                x_sb = xin.tile([KP, NCH // 2, 2, RB], BF16, tag="x_sb")
                # half-granularity loads: MM1 chunks 0-3 only wait on the
                # first half; halves (256 descriptors) still spread across
                # all 16 DMA engines. First tiles load in quarters so MM1
                # starts sooner.
                load_parts = (
                    [(a, a + 1) for a in range(4)]
                    if t <= 1
                    else [(0, 2), (2, 4)]
                )
                for lo, hi in load_parts:
                    nc.sync.dma_start(
                        out=x_sb[:, lo:hi],
                        in_=x_d[t, lo:hi].rearrange("a p b r -> p a b r"),
                    )

                # MM1: A^T [2m, RB] accumulated over the 8 L-chunks
                a_ps = ps_a.tile([M2, RB], F32, tag="a_ps")
                for c in range(NCH):
                    nc.tensor.matmul(
                        a_ps,
                        pP[:, c, :],
                        x_sb[:KC, c // 2, c % 2, :],
                        start=(c == 0),
                        stop=(c == NCH - 1),
                    )
                a_sb = apool.tile([M2, RB], BF16, tag="a_sb")
                nc.scalar.copy(a_sb, a_ps)

                # MM2 + x-add per L-chunk. GPSIMD can't read PSUM, so the
                # add work is split: 5 chunks as DVE tensor_adds from
                # PSUM, 3 chunks fold x into the PSUM accumulation via an
                # identity matmul (PE has slack) leaving ACT a plain copy.
                y_sb = yout.tile([KP, NCH // 2, 2, RB], BF16, tag="y_sb")
                for c in range(NCH):
                    fold = c in (2, 5, 7)
                    x_c = x_sb[:, c // 2, c % 2, :]
                    y_c = y_sb[:, c // 2, c % 2, :]
                    ct_ps = ps_c.tile([KP, RB], F32, tag="ct_ps")
                    # qQ free dim is host-padded with zeros beyond KC, so
                    # out partitions KC..KP-1 come out zero (defined)
                    nc.tensor.matmul(
                        ct_ps, qQ[:, c, :], a_sb, start=True, stop=not fold
                    )
                    if fold:
                        nc.tensor.matmul(
                            ct_ps,
                            ident[:KC, :],
                            x_sb[:KC, c // 2, c % 2, :],
                            start=False,
                            stop=True,
                        )
                        nc.scalar.copy(y_c, ct_ps)
                    else:
                        nc.vector.tensor_add(y_c, x_c, ct_ps)

                # half-granularity stores issued as each half's adds land,
                # alternating the ACT HWDGE ring and the GpSimd SWDGE ring
                # (SP stays load-only so stores never head-block loads).
                # The last tiles store per-quarter so the drain shrinks.
                store_parts = (
                    [(a, a + 1) for a in range(4)]
                    if t >= NT - 2
                    else [(0, 2), (2, 4)]
                )
                for lo, hi in store_parts:
                    eng = nc.scalar if lo % 2 == 0 else nc.gpsimd
                    eng.dma_start(
                        out=y_d[t, lo:hi].rearrange("a p b r -> p a b r"),
                        in_=y_sb[:, lo:hi],
                    )

    nc.compile()
    return nc


def kernel(x, fweights, fweights_im, index):
    global _nc_cache, LAST_RESULT
    x = np.asarray(x, dtype=np.float32)
    P, Q = _build_pq(fweights, fweights_im, index)
    p_host = np.ascontiguousarray(
        P.reshape(NCH, KC, M2).transpose(1, 0, 2)
    ).astype(NP_BF16)
    q_host = np.zeros((M2, NCH, KP), dtype=NP_BF16)
    q_host[:, :, :KC] = Q.reshape(M2, NCH, KC).astype(NP_BF16)

    if _nc_cache is None:
        _nc_cache = _build_bass()
    nc = _nc_cache

    xb = x.reshape(ROWS, L).astype(NP_BF16)
    in_maps = []
    for c in range(NCORES):
        xc = xb[c * R_CORE : (c + 1) * R_CORE]
        # [t, r, c4, c2, p] -> [t, c4, p, c2, r], zero-padded p: KC -> KP
        xt = np.zeros((NT, NCH // 2, KP, 2, RB), dtype=NP_BF16)
        xt[:, :, :KC] = xc.reshape(NT, RB, NCH // 2, 2, KC).transpose(
            0, 2, 4, 3, 1
        )
        in_maps.append({"x": xt, "p": p_host, "q": q_host})

    res = run_bass_kernel_spmd(
        nc, in_maps, core_ids=list(range(NCORES)), trace=TRACE
    )
    LAST_RESULT = res
    y = np.empty((ROWS, L), dtype=np.float32)
    for c in range(NCORES):
        yt = res.results[c]["y"]  # [NT, 4, KP, 2, RB] bf16
        y[c * R_CORE : (c + 1) * R_CORE] = (
            yt[:, :, :KC]
            .transpose(0, 4, 1, 3, 2)
            .reshape(R_CORE, L)
            .astype(np.float32)
        )
    return y.reshape(B, 1, E, L)


# revision 60
# speedup vs baseline: 1.0035x; 1.0035x over previous
"""Trainium2 Bass kernel for the FNO-style spectral layer.

Math: reference computes y = irfft(rfft(x) + delta) along L where delta
only touches output bins 0..63:
    delta[k] = fre[index[k]] * wr[k] + i * fim[index[k]] * wi[k]
By linearity of rfft/irfft, y = x + x @ P @ Q where
    P[n, k]      =  wr[k] * cos(2*pi*index[k]*n/L) / sqrt(L)
    P[n, 64+k]   = -wi[k] * sin(2*pi*index[k]*n/L) / sqrt(L)
    Q[k, n]      =  c_k * cos(2*pi*k*n/L)          (c_0 = 1/sqrt(L), else 2/sqrt(L))
    Q[64+k, n]   = -c_k * sin(2*pi*k*n/L)
(the jax irfft ignores the imaginary part of bin 0; row 64 of Q is zero
anyway since sin(0) == 0).

The norm rel-err budget (2e-2) is ~10x above bf16 I/O noise (~3e-3), so
everything runs in bf16: HBM traffic halves versus f32 (the kernel is
memory-bound — 45 MB/core, 90 MB per core-pair through one 716 GB/s
stack -> ~126 us floor vs ~255 us for f32).

The host uploads x PRE-TRANSPOSED per core as [NT, 4, 125, 2, RB] where
element (t, c4, p, c2, r) = x_rows[t*RB + r, (2*c4 + c2)*125 + p]. The
device then needs NO PE transposes at all:
    MM1: A^T[2m, RB] = sum_c P_c^T @ xt_c      (contract L in 8 chunks)
    MM2: corr^T chunk c = Q_c^T @ A^T          (per 125-row L chunk)
    y^T = x^T + corr^T   (5 chunks: DVE adds from PSUM; 3 chunks:
                          ACT copy PSUM->SBUF + GpSimd SBUF add)
y is stored in the same transposed tiled layout and un-permuted on the
host. The partition dim is PADDED 125 -> 128 (zero rows): the DGE
splits a DMA instruction's partition dim EVENLY across DMA engines, so
the engine count is the largest divisor of the partition count <= 16.
125 partitions -> 5 engines -> 1/3 bandwidth (measured, twice); 128
partitions -> all 16. The (4, 2) chunk split keeps descriptors at 2 KB,
512 per instruction.

Measured: ~140-146 us HW exec per 8-core run (vs 274 us for the f32
version of the same pipeline), rel err 2.3e-03 against the f32 jax
reference. DMA engines sit at ~83-85% of the 46 MB / 358 GB/s per-core
floor (~129 us); the rest is NEFF startup (~5.5 us), the end-of-kernel
semaphore drain (~4 us), and drain-phase gaps where the last tiles'
stores trail their adds. Run-to-run spread is a few us.
"""

import sys

if "/opt/trn_rl_repo" not in sys.path:
    sys.path.insert(0, "/opt/trn_rl_repo")

import ml_dtypes
import numpy as np

import concourse.bass as bass  # noqa: F401  (kept for AP helpers)
import concourse.mybir as mybir
from concourse import bacc
from concourse.bass_utils import run_bass_kernel_spmd
from concourse.masks import make_identity
from concourse.tile import TileContext

B, E, L = 4096, 22, 1000
MODES = 64
M2 = 2 * MODES                # 128
NCORES = 8
ROWS = B * E                  # 90112
R_CORE = ROWS // NCORES       # 11264
RB = 512                      # batch-rows per tile
NT = R_CORE // RB             # 22
KC = 125                      # L-chunk (partition dim), 8 * 125 = 1000
NCH = L // KC                 # 8

KP = 128                      # padded partition dim (KC zero-padded)

F32 = mybir.dt.float32
BF16 = mybir.dt.bfloat16
NP_BF16 = ml_dtypes.bfloat16

# knobs (module-level so test.py can flip them before first kernel() call)
TRACE = False
LAST_RESULT = None


def _build_pq(fweights, fweights_im, index):
    """Host-side: analysis P [L, 2m] and synthesis Q [2m, L] in float64."""
    fw = np.asarray(fweights, dtype=np.float64)
    fwi = np.asarray(fweights_im, dtype=np.float64)
    idx = np.asarray(index, dtype=np.int64)
    m = idx.shape[0]
    widx = np.concatenate([[0], np.arange(1, m) + 1])
    wr = fw[widx, 0]
    wi = fwi[widx, 0]
    n = np.arange(L, dtype=np.float64)
    ang_in = 2.0 * np.pi * np.outer(n, idx.astype(np.float64)) / L
    P = np.zeros((L, 2 * m), dtype=np.float64)
    P[:, :m] = np.cos(ang_in) * wr / np.sqrt(L)
    P[:, m:] = -np.sin(ang_in) * wi / np.sqrt(L)
    k_out = np.arange(m, dtype=np.float64)
    ang_out = 2.0 * np.pi * np.outer(k_out, n) / L
    c = np.full(m, 2.0 / np.sqrt(L))
    c[0] = 1.0 / np.sqrt(L)
    Q = np.zeros((2 * m, L), dtype=np.float64)
    Q[:m, :] = np.cos(ang_out) * c[:, None]
    Q[m:, :] = -np.sin(ang_out) * c[:, None]
    return P, Q


_nc_cache = None


def _build_bass():
    nc = bacc.Bacc(None, target_bir_lowering=False)
    x_d = nc.dram_tensor(
        "x", [NT, NCH // 2, KP, 2, RB], BF16, kind="ExternalInput"
    )
    p_d = nc.dram_tensor("p", [KC, NCH, M2], BF16, kind="ExternalInput")
    q_d = nc.dram_tensor("q", [M2, NCH, KP], BF16, kind="ExternalInput")
    y_d = nc.dram_tensor(
        "y", [NT, NCH // 2, KP, 2, RB], BF16, kind="ExternalOutput"
    )

    with TileContext(nc) as tc:
        with (
            tc.tile_pool(name="consts", bufs=1) as consts,
            tc.tile_pool(name="xin", bufs=8) as xin,
            tc.tile_pool(name="apool", bufs=3) as apool,
            tc.tile_pool(name="yout", bufs=4) as yout,
            tc.tile_pool(name="ps_a", bufs=2, space="PSUM") as ps_a,
            tc.tile_pool(name="ps_pair", bufs=2, space="PSUM") as ps_pair,
            tc.tile_pool(name="ps_one", bufs=2, space="PSUM") as ps_one,
        ):
            # params staged on the SWDGE (gpsimd) ring so the SP ring is
            # free for the first x loads
            pP = consts.tile([KC, NCH, M2], BF16)
            nc.gpsimd.dma_start(out=pP, in_=p_d[:, :, :])
            qQ = consts.tile([M2, NCH, KP], BF16)
            nc.gpsimd.dma_start(out=qQ, in_=q_d[:, :, :])
            ident = consts.tile([KP, KP], BF16)
            make_identity(nc, ident)

            for t in range(NT):
                x_sb = xin.tile([KP, NCH // 2, 2, RB], BF16, tag="x_sb")
                # half-granularity loads: MM1 chunks 0-3 only wait on the
                # first half; halves (256 descriptors) still spread across
                # all 16 DMA engines. First tiles load in quarters so MM1
                # starts sooner.
                load_parts = (
                    [(a, a + 1) for a in range(4)]
                    if t <= 1
                    else [(0, 2), (2, 4)]
                )
                for lo, hi in load_parts:
                    nc.sync.dma_start(
                        out=x_sb[:, lo:hi],
                        in_=x_d[t, lo:hi].rearrange("a p b r -> p a b r"),
                    )

                # MM1: A^T [2m, RB] accumulated over the 8 L-chunks
                a_ps = ps_a.tile([M2, RB], F32, tag="a_ps")
                for c in range(NCH):
                    nc.tensor.matmul(
                        a_ps,
                        pP[:, c, :],
                        x_sb[:KC, c // 2, c % 2, :],
                        start=(c == 0),
                        stop=(c == NCH - 1),
                    )
                a_sb = apool.tile([M2, RB], BF16, tag="a_sb")
                nc.scalar.copy(a_sb, a_ps)

                # MM2 + x-add per L-chunk. Chunk pairs aligned with the
                # (c4, c2) layout land in one 2-bank PSUM tile so a single
                # merged DVE add / ACT copy covers [KP, 2, RB] — halving
                # the per-instruction PSUM-latency + sequencer overhead.
                # Chunks 0-3: two merged DVE adds from PSUM. Chunk 4:
                # single DVE add. Chunks 5-7 fold x into the PSUM
                # accumulation via an identity matmul (PE has slack;
                # GPSIMD can't read PSUM), leaving ACT plain copies:
                # chunk 5 single, chunks 6-7 merged.
                # qQ free dim is host-padded with zeros beyond KC, so out
                # partitions KC..KP-1 come out zero (defined) everywhere.
                y_sb = yout.tile([KP, NCH // 2, 2, RB], BF16, tag="y_sb")
                for a in (0, 1):
                    ct2 = ps_pair.tile([KP, 2, RB], F32, tag="ct2")
                    for b in (0, 1):
                        nc.tensor.matmul(
                            ct2[:, b, :],
                            qQ[:, 2 * a + b, :],
                            a_sb,
                            start=True,
                            stop=True,
                        )
                    nc.vector.tensor_add(y_sb[:, a], x_sb[:, a], ct2)

                ct1 = ps_one.tile([KP, RB], F32, tag="ct1")
                nc.tensor.matmul(ct1, qQ[:, 4, :], a_sb, start=True, stop=True)
                nc.vector.tensor_add(
                    y_sb[:, 2, 0, :], x_sb[:, 2, 0, :], ct1
                )

                ct1f = ps_one.tile([KP, RB], F32, tag="ct1")
                nc.tensor.matmul(
                    ct1f, qQ[:, 5, :], a_sb, start=True, stop=False
                )
                nc.tensor.matmul(
                    ct1f, ident[:KC, :], x_sb[:KC, 2, 1, :],
                    start=False, stop=True,
                )
                nc.scalar.copy(y_sb[:, 2, 1, :], ct1f)

                ct2f = ps_pair.tile([KP, 2, RB], F32, tag="ct2")
                for b in (0, 1):
                    nc.tensor.matmul(
                        ct2f[:, b, :],
                        qQ[:, 6 + b, :],
                        a_sb,
                        start=True,
                        stop=False,
                    )
                    nc.tensor.matmul(
                        ct2f[:, b, :], ident[:KC, :], x_sb[:KC, 3, b, :],
                        start=False, stop=True,
                    )
                nc.scalar.copy(y_sb[:, 3], ct2f)

                # half-granularity stores issued as each half's adds land,
                # alternating the ACT HWDGE ring and the GpSimd SWDGE ring
                # (SP stays load-only so stores never head-block loads).
                # The last tiles store per-quarter so the drain shrinks.
                store_parts = (
                    [(a, a + 1) for a in range(4)]
                    if t >= NT - 2
                    else [(0, 2), (2, 4)]
                )
                for lo, hi in store_parts:
                    eng = nc.scalar if lo % 2 == 0 else nc.gpsimd
                    eng.dma_start(
                        out=y_d[t, lo:hi].rearrange("a p b r -> p a b r"),
                        in_=y_sb[:, lo:hi],
                    )

    nc.compile()
    return nc


def kernel(x, fweights, fweights_im, index):
    global _nc_cache, LAST_RESULT
    x = np.asarray(x, dtype=np.float32)
    P, Q = _build_pq(fweights, fweights_im, index)
    p_host = np.ascontiguousarray(
        P.reshape(NCH, KC, M2).transpose(1, 0, 2)
    ).astype(NP_BF16)
    q_host = np.zeros((M2, NCH, KP), dtype=NP_BF16)
    q_host[:, :, :KC] = Q.reshape(M2, NCH, KC).astype(NP_BF16)

    if _nc_cache is None:
        _nc_cache = _build_bass()
    nc = _nc_cache

    xb = x.reshape(ROWS, L).astype(NP_BF16)
    in_maps = []
    for c in range(NCORES):
        xc = xb[c * R_CORE : (c + 1) * R_CORE]
        # [t, r, c4, c2, p] -> [t, c4, p, c2, r], zero-padded p: KC -> KP
        xt = np.zeros((NT, NCH // 2, KP, 2, RB), dtype=NP_BF16)
        xt[:, :, :KC] = xc.reshape(NT, RB, NCH // 2, 2, KC).transpose(
            0, 2, 4, 3, 1
        )
        in_maps.append({"x": xt, "p": p_host, "q": q_host})

    res = run_bass_kernel_spmd(
        nc, in_maps, core_ids=list(range(NCORES)), trace=TRACE
    )
    LAST_RESULT = res
    y = np.empty((ROWS, L), dtype=np.float32)
    for c in range(NCORES):
        yt = res.results[c]["y"]  # [NT, 4, KP, 2, RB] bf16
        y[c * R_CORE : (c + 1) * R_CORE] = (
            yt[:, :, :KC]
            .transpose(0, 4, 1, 3, 2)
            .reshape(R_CORE, L)
            .astype(np.float32)
        )
    return y.reshape(B, 1, E, L)


# revision 61
# speedup vs baseline: 1.0863x; 1.0825x over previous
"""Trainium2 Bass kernel for the FNO-style spectral layer.

Math: reference computes y = irfft(rfft(x) + delta) along L where delta
only touches output bins 0..63:
    delta[k] = fre[index[k]] * wr[k] + i * fim[index[k]] * wi[k]
By linearity of rfft/irfft, y = x + x @ P @ Q where
    P[n, k]      =  wr[k] * cos(2*pi*index[k]*n/L) / sqrt(L)
    P[n, 64+k]   = -wi[k] * sin(2*pi*index[k]*n/L) / sqrt(L)
    Q[k, n]      =  c_k * cos(2*pi*k*n/L)          (c_0 = 1/sqrt(L), else 2/sqrt(L))
    Q[64+k, n]   = -c_k * sin(2*pi*k*n/L)
(the jax irfft ignores the imaginary part of bin 0; row 64 of Q is zero
anyway since sin(0) == 0).

The norm rel-err budget (2e-2) is ~10x above bf16 I/O noise (~3e-3), so
everything runs in bf16: HBM traffic halves versus f32 (the kernel is
memory-bound — 45 MB/core, 90 MB per core-pair through one 716 GB/s
stack -> ~126 us floor vs ~255 us for f32).

The host uploads x PRE-TRANSPOSED per core as [NT, 4, 125, 2, RB] where
element (t, c4, p, c2, r) = x_rows[t*RB + r, (2*c4 + c2)*125 + p]. The
device then needs NO PE transposes at all:
    MM1: A^T[2m, RB] = sum_c P_c^T @ xt_c      (contract L in 8 chunks)
    MM2: corr^T chunk c = Q_c^T @ A^T          (per 125-row L chunk)
    y^T = x^T + corr^T   (5 chunks: DVE adds from PSUM; 3 chunks:
                          ACT copy PSUM->SBUF + GpSimd SBUF add)
y is stored in the same transposed tiled layout and un-permuted on the
host. The partition dim is PADDED 125 -> 128 (zero rows): the DGE
splits a DMA instruction's partition dim EVENLY across DMA engines, so
the engine count is the largest divisor of the partition count <= 16.
125 partitions -> 5 engines -> 1/3 bandwidth (measured, twice); 128
partitions -> all 16. The (4, 2) chunk split keeps descriptors at 2 KB,
512 per instruction.

Measured: ~140-146 us HW exec per 8-core run (vs 274 us for the f32
version of the same pipeline), rel err 2.3e-03 against the f32 jax
reference. DMA engines sit at ~83-85% of the 46 MB / 358 GB/s per-core
floor (~129 us); the rest is NEFF startup (~5.5 us), the end-of-kernel
semaphore drain (~4 us), and drain-phase gaps where the last tiles'
stores trail their adds. Run-to-run spread is a few us.
"""

import sys

if "/opt/trn_rl_repo" not in sys.path:
    sys.path.insert(0, "/opt/trn_rl_repo")

import ml_dtypes
import numpy as np

import concourse.bass as bass  # noqa: F401  (kept for AP helpers)
import concourse.mybir as mybir
from concourse import bacc
from concourse.bass_utils import run_bass_kernel_spmd
from concourse.masks import make_identity
from concourse.tile import TileContext

B, E, L = 4096, 22, 1000
MODES = 64
M2 = 2 * MODES                # 128
NCORES = 8
ROWS = B * E                  # 90112
R_CORE = ROWS // NCORES       # 11264
RB = 512                      # batch-rows per tile
NT = R_CORE // RB             # 22
KC = 125                      # L-chunk (partition dim), 8 * 125 = 1000
NCH = L // KC                 # 8

KP = 128                      # padded partition dim (KC zero-padded)

F32 = mybir.dt.float32
BF16 = mybir.dt.bfloat16
NP_BF16 = ml_dtypes.bfloat16

# knobs (module-level so test.py can flip them before first kernel() call)
TRACE = False
LAST_RESULT = None


def _build_pq(fweights, fweights_im, index):
    """Host-side: analysis P [L, 2m] and synthesis Q [2m, L] in float64."""
    fw = np.asarray(fweights, dtype=np.float64)
    fwi = np.asarray(fweights_im, dtype=np.float64)
    idx = np.asarray(index, dtype=np.int64)
    m = idx.shape[0]
    widx = np.concatenate([[0], np.arange(1, m) + 1])
    wr = fw[widx, 0]
    wi = fwi[widx, 0]
    n = np.arange(L, dtype=np.float64)
    ang_in = 2.0 * np.pi * np.outer(n, idx.astype(np.float64)) / L
    P = np.zeros((L, 2 * m), dtype=np.float64)
    P[:, :m] = np.cos(ang_in) * wr / np.sqrt(L)
    P[:, m:] = -np.sin(ang_in) * wi / np.sqrt(L)
    k_out = np.arange(m, dtype=np.float64)
    ang_out = 2.0 * np.pi * np.outer(k_out, n) / L
    c = np.full(m, 2.0 / np.sqrt(L))
    c[0] = 1.0 / np.sqrt(L)
    Q = np.zeros((2 * m, L), dtype=np.float64)
    Q[:m, :] = np.cos(ang_out) * c[:, None]
    Q[m:, :] = -np.sin(ang_out) * c[:, None]
    return P, Q


_nc_cache = None


def _build_bass():
    nc = bacc.Bacc(None, target_bir_lowering=False)
    x_d = nc.dram_tensor(
        "x", [NT, NCH // 2, KP, 2, RB], BF16, kind="ExternalInput"
    )
    p_d = nc.dram_tensor("p", [KC, NCH, M2], BF16, kind="ExternalInput")
    q_d = nc.dram_tensor("q", [M2, NCH, KP], BF16, kind="ExternalInput")
    y_d = nc.dram_tensor(
        "y", [NT, NCH // 2, KP, 2, RB], BF16, kind="ExternalOutput"
    )

    with TileContext(nc) as tc:
        with (
            tc.tile_pool(name="consts", bufs=1) as consts,
            tc.tile_pool(name="xin", bufs=8) as xin,
            tc.tile_pool(name="apool", bufs=3) as apool,
            tc.tile_pool(name="yout", bufs=4) as yout,
            tc.tile_pool(name="ps_a", bufs=3, space="PSUM") as ps_a,
            tc.tile_pool(name="ps_c", bufs=5, space="PSUM") as ps_c,
        ):
            # params staged on the SWDGE (gpsimd) ring so the SP ring is
            # free for the first x loads
            pP = consts.tile([KC, NCH, M2], BF16)
            nc.gpsimd.dma_start(out=pP, in_=p_d[:, :, :])
            qQ = consts.tile([M2, NCH, KP], BF16)
            nc.gpsimd.dma_start(out=qQ, in_=q_d[:, :, :])
            ident = consts.tile([KP, KP], BF16)
            make_identity(nc, ident)

            for t in range(NT):
                x_sb = xin.tile([KP, NCH // 2, 2, RB], BF16, tag="x_sb")
                # half-granularity loads: MM1 chunks 0-3 only wait on the
                # first half; halves (256 descriptors) still spread across
                # all 16 DMA engines. First tiles load in quarters so MM1
                # starts sooner.
                load_parts = (
                    [(a, a + 1) for a in range(4)]
                    if t <= 1
                    else [(0, 2), (2, 4)]
                )
                for lo, hi in load_parts:
                    nc.sync.dma_start(
                        out=x_sb[:, lo:hi],
                        in_=x_d[t, lo:hi].rearrange("a p b r -> p a b r"),
                    )

                # MM1: A^T [2m, RB] accumulated over the 8 L-chunks
                a_ps = ps_a.tile([M2, RB], F32, tag="a_ps")
                for c in range(NCH):
                    nc.tensor.matmul(
                        a_ps,
                        pP[:, c, :],
                        x_sb[:KC, c // 2, c % 2, :],
                        start=(c == 0),
                        stop=(c == NCH - 1),
                    )
                a_sb = apool.tile([M2, RB], BF16, tag="a_sb")
                nc.scalar.copy(a_sb, a_ps)

                # MM2 + x-add per L-chunk. GPSIMD can't read PSUM, so the
                # add work is split: 5 chunks as DVE tensor_adds from
                # PSUM, 3 chunks fold x into the PSUM accumulation via an
                # identity matmul (PE has slack) leaving ACT a plain copy.
                y_sb = yout.tile([KP, NCH // 2, 2, RB], BF16, tag="y_sb")
                for c in range(NCH):
                    fold = c in (2, 5, 7)
                    x_c = x_sb[:, c // 2, c % 2, :]
                    y_c = y_sb[:, c // 2, c % 2, :]
                    ct_ps = ps_c.tile([KP, RB], F32, tag="ct_ps")
                    # qQ free dim is host-padded with zeros beyond KC, so
                    # out partitions KC..KP-1 come out zero (defined)
                    nc.tensor.matmul(
                        ct_ps, qQ[:, c, :], a_sb, start=True, stop=not fold
                    )
                    if fold:
                        nc.tensor.matmul(
                            ct_ps,
                            ident[:KC, :],
                            x_sb[:KC, c // 2, c % 2, :],
                            start=False,
                            stop=True,
                        )
                        nc.scalar.copy(y_c, ct_ps)
                    else:
                        nc.vector.tensor_add(y_c, x_c, ct_ps)

                # half-granularity stores issued as each half's adds land,
                # alternating the ACT HWDGE ring and the GpSimd SWDGE ring
                # (SP stays load-only so stores never head-block loads).
                # The last tiles store per-quarter so the drain shrinks.
                store_parts = (
                    [(a, a + 1) for a in range(4)]
                    if t >= NT - 2
                    else [(0, 2), (2, 4)]
                )
                for lo, hi in store_parts:
                    eng = nc.scalar if lo % 2 == 0 else nc.gpsimd
                    eng.dma_start(
                        out=y_d[t, lo:hi].rearrange("a p b r -> p a b r"),
                        in_=y_sb[:, lo:hi],
                    )

    nc.compile()
    return nc


def kernel(x, fweights, fweights_im, index):
    global _nc_cache, LAST_RESULT
    x = np.asarray(x, dtype=np.float32)
    P, Q = _build_pq(fweights, fweights_im, index)
    p_host = np.ascontiguousarray(
        P.reshape(NCH, KC, M2).transpose(1, 0, 2)
    ).astype(NP_BF16)
    q_host = np.zeros((M2, NCH, KP), dtype=NP_BF16)
    q_host[:, :, :KC] = Q.reshape(M2, NCH, KC).astype(NP_BF16)

    if _nc_cache is None:
        _nc_cache = _build_bass()
    nc = _nc_cache

    xb = x.reshape(ROWS, L).astype(NP_BF16)
    in_maps = []
    for c in range(NCORES):
        xc = xb[c * R_CORE : (c + 1) * R_CORE]
        # [t, r, c4, c2, p] -> [t, c4, p, c2, r], zero-padded p: KC -> KP
        xt = np.zeros((NT, NCH // 2, KP, 2, RB), dtype=NP_BF16)
        xt[:, :, :KC] = xc.reshape(NT, RB, NCH // 2, 2, KC).transpose(
            0, 2, 4, 3, 1
        )
        in_maps.append({"x": xt, "p": p_host, "q": q_host})

    res = run_bass_kernel_spmd(
        nc, in_maps, core_ids=list(range(NCORES)), trace=TRACE
    )
    LAST_RESULT = res
    y = np.empty((ROWS, L), dtype=np.float32)
    for c in range(NCORES):
        yt = res.results[c]["y"]  # [NT, 4, KP, 2, RB] bf16
        y[c * R_CORE : (c + 1) * R_CORE] = (
            yt[:, :, :KC]
            .transpose(0, 4, 1, 3, 2)
            .reshape(R_CORE, L)
            .astype(np.float32)
        )
    return y.reshape(B, 1, E, L)
